# revision 13
# baseline (speedup 1.0000x reference)
"""TreeLSTM (complete binary tree, S=255, B=64) on 8 trn2 NeuronCores.

Sharding: data-parallel over batch (8 examples per core); every core holds the
full weights. No collectives needed (trees independent).

Per-core pipeline (v3):
  1. Host gathers the embedding rows (bf16, bias channel at feat 200) and lays
     them out feature-major, so the kernel starts with plain dense DMAs on the
     two HWDGE rings (sync: embeddings, scalar: weights) -- no gpsimd gather,
     no PE transposes.
  2. wx_iou = W_iou^T emb^T (+b) via bf16 matmuls with 128-row K chunks; leaf
     gates consume the PSUM directly (fused); internal wxf held in SBUF
     (PSUM -> SBUF move on DVE, not ACT).
  3. 8 tree levels, deepest first: child-sum via strided even/odd adds,
     forget gates, iou gates, cell/hidden updates.
  4. Head: psum = [h_root.wz, -h_root.wz]; one Softplus ACT (scale=-1,
     per-partition bias -+bz) gives sp(-z), sp(z); host negates/transposes:
     log_softmax = [-sp(-z), -sp(z)].
"""

import os
import sys
import types

import numpy as np


def _install_axon_hook():
    """Register the NTFF profile hook so BASS_TRACE=1 tracing works."""
    try:
        import antenv

        if "antenv.axon_hooks" in sys.modules:
            return
        hooks = types.ModuleType("antenv.axon_hooks")
        hooks._hook = None

        def set_axon_ntff_profile_hook(h):
            hooks._hook = h

        def get_axon_ntff_profile_hook():
            return hooks._hook

        hooks.set_axon_ntff_profile_hook = set_axon_ntff_profile_hook
        hooks.get_axon_ntff_profile_hook = get_axon_ntff_profile_hook
        sys.modules["antenv.axon_hooks"] = hooks
        antenv.axon_hooks = hooks
        try:
            from trn_agent_boot.trn_boot import _ntff_profile_via_ctypes

            set_axon_ntff_profile_hook(
                _ntff_profile_via_ctypes("/opt/axon/libaxon_pjrt.so")
            )
        except Exception:
            pass
    except Exception:
        pass


_install_axon_hook()

from contextlib import ExitStack  # noqa: E402

import ml_dtypes  # noqa: E402

import concourse.bacc as bacc  # noqa: E402
import concourse.mybir as mybir  # noqa: E402
import concourse.tile as tile  # noqa: E402

from concourse.bass_utils import run_bass_kernel_spmd  # noqa: E402

F32 = mybir.dt.float32
BF16 = mybir.dt.bfloat16
AF = mybir.ActivationFunctionType
OP = mybir.AluOpType
BF16NP = ml_dtypes.bfloat16

E, HID, NCLS = 200, 300, 2
B, S, V = 64, 255, 32000
NCORES, BL = 8, 8
CH = 100  # U-matmul K chunk (H feature rows per chunk)
CM = 128  # M chunk physical width (100 logical + 28 zero pad -> FWL)
KE, KH = 2, 3  # K-chunks: emb side 2x128; H side 3x100
NLEAF = 1024  # 128 leaf nodes * 8 batch
NINT = 1016  # 127 internal nodes * 8
NINTP = 1024  # padded: [pad(8), nodes 0..126]

TAPS = bool(int(os.environ.get("BASS_KERNEL_TAPS", "0")))


def _build():
    nc = bacc.Bacc(
        "TRN2", target_bir_lowering=False, debug=False, num_devices=NCORES,
        enable_asserts=False,
    )
    embl_d = nc.dram_tensor("embl", [128, 2, KE, 512], BF16, kind="ExternalInput")
    embi_d = nc.dram_tensor("embi", [128, KE, NINTP], BF16, kind="ExternalInput")
    # wbig: wiou gates 0..2 + wf as "gate" 3 -- one DMA
    wbig_d = nc.dram_tensor("wbig", [128, 4, KE, KH * CM], BF16, kind="ExternalInput")
    # ubig: per k-chunk [uiou g0|g1|g2 (3*384), uf (384), wz (2)] -- one DMA
    ubig_d = nc.dram_tensor("ubig", [CH, KH, 4 * KH * CM + 2], BF16, kind="ExternalInput")
    bias_d = nc.dram_tensor("bias", [2, 1], F32, kind="ExternalInput")
    out_d = nc.dram_tensor("out", [2, BL], F32, kind="ExternalOutput")

    taps = {}
    if TAPS:
        taps["wxf"] = nc.dram_tensor("tap_wxf", [CM, KH, NINTP], BF16, kind="ExternalOutput")
        taps["H7"] = nc.dram_tensor("tap_H7", [CM, KH, NLEAF], BF16, kind="ExternalOutput")
        taps["C7"] = nc.dram_tensor("tap_C7", [CM, KH, NLEAF], BF16, kind="ExternalOutput")
        taps["H5"] = nc.dram_tensor("tap_H5", [CM, KH, 256], BF16, kind="ExternalOutput")
        taps["H0"] = nc.dram_tensor("tap_H0", [CM, KH, 8], BF16, kind="ExternalOutput")

    with tile.TileContext(nc) as tc, ExitStack() as ctx:
        const = ctx.enter_context(tc.tile_pool(name="const", bufs=1))
        acts = ctx.enter_context(tc.tile_pool(name="acts", bufs=1))
        tr = ctx.enter_context(tc.tile_pool(name="tr", bufs=1))
        sm = ctx.enter_context(tc.tile_pool(name="sm", bufs=1))

        # ---- input DMAs split across both HWDGE rings by consumption
        # order. Sync ring carries the critical first-leaf inputs; the
        # scalar ring first runs the ACT-table warmup (so the table loads
        # overlap the sync-ring transfers), then issues the later inputs.
        embl_t = acts.tile([128, 2, KE, 512], BF16, tag="embl", name="embl")
        wbig_t = const.tile([128, 4, KE, KH * CM], BF16, tag="wbig", name="wbig")
        embi_t = acts.tile([128, KE, NINTP], BF16, tag="embi", name="embi")
        ubig_t = const.tile([CH, KH, 4 * KH * CM + 2], BF16, tag="ubig", name="ubig")
        bias_t = const.tile([2, 1], F32, tag="bias", name="bias")
        nc.sync.dma_start(embl_t[:, 0], embl_d[:, 0])
        nc.sync.dma_start(wbig_t[:, 0], wbig_d[:, 0])
        nc.sync.dma_start(wbig_t[:, 2], wbig_d[:, 2])
        nc.sync.dma_start(embl_t[:, 1], embl_d[:, 1])

        ones_t = const.tile([1, BL], BF16)
        nc.vector.memset(ones_t[:], 1.0)
        dummy_t = const.tile([1, BL], BF16)
        nc.scalar.activation(dummy_t[:], ones_t[:], AF.Sigmoid)
        nc.scalar.dma_start(wbig_t[:, 1], wbig_d[:, 1])
        nc.scalar.dma_start(wbig_t[:, 3], wbig_d[:, 3])
        nc.scalar.dma_start(embi_t[:], embi_d[:])
        nc.scalar.dma_start(ubig_t[:], ubig_d[:])
        nc.scalar.dma_start(bias_t[:], bias_d[:])

        def wiou_ap(g, k, m):  # stationary [128, 128]
            return wbig_t[:, g, k, CM * m : CM * (m + 1)]

        def wf_ap(k, m):
            return wbig_t[:, 3, k, CM * m : CM * (m + 1)]

        def uiou_ap(g, k, m):  # stationary [100, 128]
            return ubig_t[:, k, KH * CM * g + CM * m : KH * CM * g + CM * (m + 1)]

        def uf_ap(k, m):
            return ubig_t[:, k, 3 * KH * CM + CM * m : 3 * KH * CM + CM * (m + 1)]

        def wz_ap(k):  # stationary [100, 2]
            return ubig_t[:, k, 4 * KH * CM : 4 * KH * CM + 2]

        # ---- PE warm-up on a memset tile: release the HAM clock gate
        # (PE defaults to 1.2GHz; ~3.4us of sustained activity unlocks
        # 2.4GHz) while the input DMAs stream in.
        wtile = const.tile([128, 512], BF16, tag="wtile", name="wtile")
        nc.vector.memset(wtile[:], 0.0)

        # leaf gate tiles [CH, 3(mchunk), 128(node), 8(batch)]
        gi7 = acts.tile([CM, 3, 128, 8], BF16, tag="gi7")
        go7 = acts.tile([CM, 3, 128, 8], BF16, tag="go7")
        gu7 = acts.tile([CM, 3, 128, 8], BF16, tag="gu7")
        leaf_g = (gi7, go7, gu7)
        wxf_t = acts.tile([CM, KH, NINTP], BF16, tag="wxf")

        with ExitStack() as ps1:
            ps_wx = ps1.enter_context(
                tc.tile_pool(name="ps_wx", bufs=2, space="PSUM")
            )

            # Bridge PE activity from engine-start to the first leaf matmul
            # (wiou g0 + embl half 0 land ~2us in) so the HAM busy window
            # stays warm and unlocks 2.4GHz ASAP.
            warm_ps = ps_wx.tile([CM, 3, 64, 8], F32, tag="ps_wx", name="warm")
            for w in range(6):
                nc.tensor.matmul(
                    warm_ps[:, w % 3, :, :],
                    wtile[:, 0:CM],
                    wtile[:],
                    start=(w < 3),
                    stop=(w >= 3),
                    skip_group_check=True,
                )

            # ---- leaf gates fused with wx matmuls (bias folded into W row
            # 200 x emb bias channel == 1). One merged [CH, 3, 512] psum per
            # (half, gate) -> one ACT per gate half.
            for nt in range(2):
                for g in (0, 2, 1):
                    ps = ps_wx.tile([CM, 3, 64, 8], F32, tag="ps_wx", name="psw")
                    for m in range(3):
                        for k in range(KE):
                            nc.tensor.matmul(
                                ps[:, m, :, :],
                                wiou_ap(g, k, m),
                                embl_t[:, nt, k, :],
                                start=(k == 0),
                                stop=(k == KE - 1),
                            )
                    nc.scalar.activation(
                        leaf_g[g][:, :, 64 * nt : 64 * (nt + 1), :],
                        ps[:],
                        AF.Tanh if g == 2 else AF.Sigmoid,
                    )

            # ---- internal wxf (level 6 consumes it first; deep half first)
            for nt in (1, 0):
                ps = ps_wx.tile([CM, 3, 512], F32, tag="ps_wx", name="psw")
                for m in range(KH):
                    for k in range(KE):
                        nc.tensor.matmul(
                            ps[:, m, :],
                            wf_ap(k, m),
                            embi_t[:, k, 512 * nt : 512 * (nt + 1)],
                            start=(k == 0),
                            stop=(k == KE - 1),
                        )
                nc.vector.tensor_copy(
                    wxf_t[:, :, 512 * nt : 512 * (nt + 1)], ps[:]
                )

        # ---- leaf cell/hidden ----
        H = {}
        C = {}
        C[7] = acts.tile([CM, KH, 128, 8], BF16, tag="C7", name="C7")
        H[7] = acts.tile([CM, KH, 128, 8], BF16, tag="H7", name="H7")
        for h in range(2):
            nsl = slice(64 * h, 64 * (h + 1))
            nc.vector.tensor_mul(
                C[7][:, :, nsl, :], gi7[:, :, nsl, :], gu7[:, :, nsl, :]
            )
            th7 = tr.tile([CM, KH, 64, 8], BF16, tag="th", name="th7")
            nc.scalar.activation(th7[:], C[7][:, :, nsl, :], AF.Tanh)
            nc.vector.tensor_mul(
                H[7][:, :, nsl, :], go7[:, :, nsl, :], th7[:]
            )

        # ---- tree levels 6..0 ----
        with ExitStack() as ps2:
            ps_uf = ps2.enter_context(
                tc.tile_pool(name="ps_uf", bufs=2, space="PSUM")
            )
            ps_io = ps2.enter_context(
                tc.tile_pool(name="ps_io", bufs=2, space="PSUM")
            )
            for d in range(6, -1, -1):
                npar = 1 << d
                n = npar * 8
                off = 8 * (1 << d)  # internal idx layout: [pad(8), nodes]
                Hch, Cch = H[d + 1], C[d + 1]

                # iou psum prologue for i and u gates: W_iou emb[par] + bias
                # (o-gate psum is allocated later; pool bufs=2 recycles).
                def iou_prologue(g):
                    ps = ps_io.tile(
                        [CM, KH, 512], F32, tag="ps_io", name="psg"
                    )
                    for m in range(KH):
                        for k2 in range(KE):
                            nc.tensor.matmul(
                                ps[:, m, 0:n],
                                wiou_ap(g, k2, m),
                                embi_t[:, k2, off : off + n],
                                start=(k2 == 0),
                                stop=False,
                            )
                    return ps

                pss = {0: iou_prologue(0), 2: iou_prologue(2)}

                hsum = tr.tile([CM, KH, npar, 8], BF16, tag="hsum", name="hsum")
                for k in range(KH):
                    nc.vector.tensor_add(
                        hsum[:, k], Hch[:, k, 0::2, :], Hch[:, k, 1::2, :]
                    )

                # forget gates: psum preloaded with wxf (vector cast), U_f
                # matmuls accumulate; f = sigma(psum). [CH, KH, npar, 2, 8].
                f2 = tr.tile([CM, KH, npar, 2, 8], BF16, tag="f2", name="f2")
                nchunk = max(n // 256, 1)  # child cols per psum <= 512
                cpn = npar // nchunk
                for m in range(KH):
                    for h in range(nchunk):
                        ps = ps_uf.tile(
                            [CM, cpn, 2, 8], F32, tag="ps_uf", name="psu"
                        )
                        wxfs = wxf_t[
                            :, m, off + 8 * cpn * h : off + 8 * cpn * (h + 1)
                        ].rearrange("p (n e) -> p n e", e=8)
                        nc.vector.tensor_copy(ps[:, :, 0, :], wxfs)
                        nc.vector.tensor_copy(ps[:, :, 1, :], wxfs)
                        for k in range(KH):
                            nc.tensor.matmul(
                                ps[:],
                                uf_ap(k, m),
                                Hch[0:CH, k, 2 * cpn * h : 2 * cpn * (h + 1), :],
                                start=False,
                                stop=(k == KH - 1),
                                skip_group_check=True,
                            )
                        nc.scalar.activation(
                            f2[:, m, cpn * h : cpn * (h + 1), :, :],
                            ps[:],
                            AF.Sigmoid,
                        )
                # fc = sum over the two children of f * C (vector, bf16)
                tci2 = tr.tile([CM, KH, npar, 2, 8], BF16, tag="tci2", name="tci2")
                cview = Cch[:].rearrange("p k (n two) b -> p k n two b", two=2)
                fc = tr.tile([CM, KH, npar, 8], BF16, tag="fc", name="fc")
                nh2 = 2 if npar >= 32 else 1
                hp2 = npar // nh2
                for q in range(nh2):
                    s_ = slice(hp2 * q, hp2 * (q + 1))
                    nc.vector.tensor_mul(
                        tci2[:, :, s_, :, :], f2[:, :, s_, :, :],
                        cview[:, :, s_, :, :],
                    )
                    nc.vector.tensor_add(
                        fc[:, :, s_, :], tci2[:, :, s_, 0, :],
                        tci2[:, :, s_, 1, :],
                    )

                # iou gates: psum = W_iou emb[par] + b + U_iou hsum; one ACT
                # per gate over all three m-chunks.
                gates = {}
                for g in (0, 2, 1):
                    gt = tr.tile(
                        [CM, KH, npar, 8], BF16, tag=f"g{g}", name="gt"
                    )
                    ps = pss.get(g)
                    if ps is None:
                        ps = iou_prologue(g)
                    for m in range(KH):
                        for k in range(KH):
                            nc.tensor.matmul(
                                ps[:, m, 0:n],
                                uiou_ap(g, k, m),
                                hsum[0:CH, k, :, :],
                                start=False,
                                stop=(k == KH - 1),
                            )
                    nc.scalar.activation(
                        gt[:],
                        ps[:, :, 0:n].rearrange("p m (c b) -> p m c b", b=8),
                        AF.Tanh if g == 2 else AF.Sigmoid,
                    )
                    gates[g] = gt
                gi, go, gu = gates[0], gates[1], gates[2]

                # HAM keep-warm fillers: run while vector/scalar do the
                # level tail; next level's matmuls then start at 2.4GHz.
                fill_ps = ps_uf.tile(
                    [CM, 512], F32, tag="ps_uf", name="fill"
                )
                for w in range(8 if npar >= 8 else 4):
                    nc.tensor.matmul(
                        fill_ps[:],
                        wtile[:, 0:CM],
                        wtile[:],
                        start=(w == 0),
                        stop=False,
                        skip_group_check=True,
                    )

                tci = tr.tile([CM, KH, npar, 8], BF16, tag="tci", name="tci")
                C[d] = acts.tile(
                    [CM, KH, npar, 8], BF16, tag=f"C{d}", name=f"C{d}"
                )
                sc = tr.tile([CM, KH, npar, 8], BF16, tag="sc", name="sc")
                H[d] = acts.tile(
                    [CM, KH, npar, 8], BF16, tag=f"H{d}", name=f"H{d}"
                )
                # split the elementwise tail into node-halves so the
                # vector/scalar chain pipelines (big levels only).
                nh = 2 if npar >= 32 else 1
                hp = npar // nh
                for q in range(nh):
                    s_ = slice(hp * q, hp * (q + 1))
                    nc.vector.tensor_mul(
                        tci[:, :, s_, :], gi[:, :, s_, :], gu[:, :, s_, :]
                    )
                    nc.vector.tensor_add(
                        C[d][:, :, s_, :], tci[:, :, s_, :], fc[:, :, s_, :]
                    )
                    nc.scalar.activation(
                        sc[:, :, s_, :], C[d][:, :, s_, :], AF.Tanh
                    )
                    nc.vector.tensor_mul(
                        H[d][:, :, s_, :], go[:, :, s_, :], sc[:, :, s_, :]
                    )

            # ---- head: psum rows = [z', -z'] (z = z' + bz);
            # log_softmax = [ln s(z), ln s(-z)] -> sigmoid ACT (bias
            # [+bz, -bz]) then Ln ACT (natural_log table), transpose on host.
            ps = ps_io.tile([2, BL], F32, tag="ps_io", name="pshead")
            for k in range(KH):
                nc.tensor.matmul(
                    ps[:],
                    wz_ap(k),
                    H[0][0:CH, k, 0, :],
                    start=(k == 0),
                    stop=(k == KH - 1),
                )
            sg_t = sm.tile([2, BL], F32, tag="sg")
            nc.scalar.activation(sg_t[:], ps[:], AF.Sigmoid, bias=bias_t[:])
            ln_t = sm.tile([2, BL], F32, tag="ln")
            nc.scalar.activation(ln_t[:], sg_t[:], AF.Ln)
            nc.sync.dma_start(out_d[:], ln_t[:])

        # ---- debug taps ----
        if TAPS:
            nc.sync.dma_start(taps["wxf"][:], wxf_t[:])
            nc.sync.dma_start(taps["H7"][:], H[7][:])
            nc.sync.dma_start(taps["C7"][:], C[7][:])
            nc.sync.dma_start(taps["H5"][:], H[5][:])
            nc.sync.dma_start(taps["H0"][:], H[0][:])

    nc.compile()
    return nc


_CACHE = {}


def _get_nc():
    if "nc" not in _CACHE:
        _CACHE["nc"] = _build()
    return _CACHE["nc"]


def _mpad(w, cols):
    """Pad each 100-wide output chunk of w [rows, cols] to 128 (zero cols)
    so stationary tiles are full 128 columns (FWL eligible)."""
    rows = w.shape[0]
    nm = cols // CH
    p = np.zeros((rows, nm, CM), np.float32)
    p[:, :, :CH] = w.reshape(rows, nm, CH)
    return p.reshape(rows, nm * CM)


def kernel(x, parent, depth, embed, W_iou, U_iou, b_iou, W_f, U_f, b_f,
           W_out, b_out):
    x = np.asarray(x)
    embed = np.asarray(embed, dtype=np.float32)
    # feature-major bf16 embedding table with bias channel at row E
    embT = np.zeros((256, V), BF16NP)
    embT[:E] = embed.T.astype(BF16NP)
    embT[E] = 1.0

    W_out = np.asarray(W_out, np.float32)
    b_out = np.asarray(b_out, np.float32)
    wz = W_out[:, 0] - W_out[:, 1]  # [300]
    bz = float(b_out[0] - b_out[1])
    wz2 = np.stack([wz, -wz], axis=1)  # [300, 2]

    wiou_b = np.vstack(
        [np.asarray(W_iou, np.float32), np.asarray(b_iou, np.float32)[None]]
    )  # [201, 900]
    wf_b = np.vstack(
        [np.asarray(W_f, np.float32), np.asarray(b_f, np.float32)[None]]
    )  # [201, 300]

    def kpad(w, rows_to):
        p = np.zeros((rows_to, w.shape[1]), np.float32)
        p[: w.shape[0]] = w
        return p

    wiou_p = kpad(_mpad(wiou_b, 3 * HID), 256)  # [256, 1152]
    wf_p = kpad(_mpad(wf_b, HID), 256)  # [256, 384]
    uiou_p = _mpad(np.asarray(U_iou, np.float32), 3 * HID)  # [300, 1152]
    uf_p = _mpad(np.asarray(U_f, np.float32), HID)  # [300, 384]

    # wbig [128, 4(g), KE, 384]: wiou gates 0..2, wf as gate 3
    wiou_r = wiou_p.reshape(2, 128, 3, KH * CM).transpose(1, 2, 0, 3)
    wf_r = wf_p.reshape(2, 128, 1, KH * CM).transpose(1, 2, 0, 3)
    wbig = np.concatenate([wiou_r, wf_r], axis=1)  # [128, 4, 2, 384]
    # ubig [100, KH(k), 4*384+2]: per k: [uiou g0|g1|g2, uf, wz]
    uiou_r = uiou_p.reshape(3, 100, 3, KH * CM).transpose(1, 0, 2, 3)  # [100,k,g,384]
    uiou_r = uiou_r.reshape(100, 3, 3 * KH * CM)
    uf_r = uf_p.reshape(3, 100, KH * CM).transpose(1, 0, 2)  # [100, k, 384]
    wz_r = wz2.reshape(3, 100, 2).transpose(1, 0, 2)  # [100, k, 2]
    ubig = np.concatenate([uiou_r, uf_r, wz_r], axis=2)  # [100, 3, 1538]
    shared = {
        "wbig": np.ascontiguousarray(wbig).astype(BF16NP),
        "ubig": np.ascontiguousarray(ubig).astype(BF16NP),
        "bias": np.array([[bz], [-bz]], np.float32),
    }
    in_maps = []
    for c in range(NCORES):
        xc = x[:, BL * c : BL * (c + 1)]  # [255, 8]
        leaf = np.ascontiguousarray(xc[127:255]).reshape(-1)  # 1024
        internal = np.concatenate(
            [np.zeros(8, np.int64), np.ascontiguousarray(xc[0:127]).reshape(-1)]
        )
        im = dict(shared)
        im["embl"] = np.ascontiguousarray(
            embT[:, leaf].reshape(2, 128, 2, 512).transpose(1, 2, 0, 3)
        )
        im["embi"] = np.ascontiguousarray(
            embT[:, internal].reshape(2, 128, NINTP).transpose(1, 0, 2)
        )
        in_maps.append(im)

    nc = _get_nc()
    res = run_bass_kernel_spmd(nc, in_maps, core_ids=list(range(NCORES)))
    kernel._last = res
    out = np.concatenate(
        [np.asarray(res.results[c]["out"]).T for c in range(NCORES)], axis=0
    )
    return np.ascontiguousarray(out.astype(np.float32))


kernel._last = None


# revision 18
# speedup vs baseline: 1.0543x; 1.0543x over previous
"""TreeLSTM (complete binary tree, S=255, B=64) on 8 trn2 NeuronCores.

Sharding: data-parallel over batch (8 examples per core); every core holds the
full weights. No collectives needed (trees independent).

Per-core pipeline (v3):
  1. Host gathers the embedding rows (bf16, bias channel at feat 200) and lays
     them out feature-major, so the kernel starts with plain dense DMAs on the
     two HWDGE rings (sync: embeddings, scalar: weights) -- no gpsimd gather,
     no PE transposes.
  2. wx_iou = W_iou^T emb^T (+b) via bf16 matmuls with 128-row K chunks; leaf
     gates consume the PSUM directly (fused); internal wxf held in SBUF
     (PSUM -> SBUF move on DVE, not ACT).
  3. 8 tree levels, deepest first: child-sum via strided even/odd adds,
     forget gates, iou gates, cell/hidden updates.
  4. Head: psum = [h_root.wz, -h_root.wz]; one Softplus ACT (scale=-1,
     per-partition bias -+bz) gives sp(-z), sp(z); host negates/transposes:
     log_softmax = [-sp(-z), -sp(z)].
"""

import os
import sys
import types

import numpy as np


def _install_axon_hook():
    """Register the NTFF profile hook so BASS_TRACE=1 tracing works."""
    try:
        import antenv

        if "antenv.axon_hooks" in sys.modules:
            return
        hooks = types.ModuleType("antenv.axon_hooks")
        hooks._hook = None

        def set_axon_ntff_profile_hook(h):
            hooks._hook = h

        def get_axon_ntff_profile_hook():
            return hooks._hook

        hooks.set_axon_ntff_profile_hook = set_axon_ntff_profile_hook
        hooks.get_axon_ntff_profile_hook = get_axon_ntff_profile_hook
        sys.modules["antenv.axon_hooks"] = hooks
        antenv.axon_hooks = hooks
        try:
            from trn_agent_boot.trn_boot import _ntff_profile_via_ctypes

            set_axon_ntff_profile_hook(
                _ntff_profile_via_ctypes("/opt/axon/libaxon_pjrt.so")
            )
        except Exception:
            pass
    except Exception:
        pass


_install_axon_hook()

from contextlib import ExitStack  # noqa: E402

import ml_dtypes  # noqa: E402

import concourse.bacc as bacc  # noqa: E402
import concourse.mybir as mybir  # noqa: E402
import concourse.tile as tile  # noqa: E402

from concourse.bass_utils import run_bass_kernel_spmd  # noqa: E402

F32 = mybir.dt.float32
BF16 = mybir.dt.bfloat16
AF = mybir.ActivationFunctionType
OP = mybir.AluOpType
BF16NP = ml_dtypes.bfloat16

E, HID, NCLS = 200, 300, 2
B, S, V = 64, 255, 32000
NCORES, BL = 8, 8
CH = 100  # U-matmul K chunk (H feature rows per chunk)
CM = 128  # M chunk physical width (100 logical + 28 zero pad -> FWL)
KE, KH = 2, 3  # K-chunks: emb side 2x128; H side 3x100
NLEAF = 1024  # 128 leaf nodes * 8 batch
NINT = 1016  # 127 internal nodes * 8
NINTP = 1024  # padded: [pad(8), nodes 0..126]

TAPS = bool(int(os.environ.get("BASS_KERNEL_TAPS", "0")))


def _build():
    nc = bacc.Bacc(
        "TRN2", target_bir_lowering=False, debug=False, num_devices=NCORES,
        enable_asserts=False,
    )
    embl_d = nc.dram_tensor("embl", [128, 2, KE, 512], BF16, kind="ExternalInput")
    embi_d = nc.dram_tensor("embi", [128, KE, NINTP], BF16, kind="ExternalInput")
    # wbig: wiou gates 0..2 + wf as "gate" 3 -- one DMA
    wbig_d = nc.dram_tensor("wbig", [128, 4, KE, KH * CM], BF16, kind="ExternalInput")
    # ubig: per k-chunk [uiou g0|g1|g2 (3*384), uf (384), wz (2)] -- one DMA
    ubig_d = nc.dram_tensor("ubig", [CH, KH, 4 * KH * CM + 2], BF16, kind="ExternalInput")
    bias_d = nc.dram_tensor("bias", [2, 1], F32, kind="ExternalInput")
    out_d = nc.dram_tensor("out", [2, BL], F32, kind="ExternalOutput")

    taps = {}
    if TAPS:
        taps["wxf"] = nc.dram_tensor("tap_wxf", [CM, KH, NINTP], BF16, kind="ExternalOutput")
        taps["H7"] = nc.dram_tensor("tap_H7", [CM, KH, NLEAF], BF16, kind="ExternalOutput")
        taps["C7"] = nc.dram_tensor("tap_C7", [CM, KH, NLEAF], BF16, kind="ExternalOutput")
        taps["H5"] = nc.dram_tensor("tap_H5", [CM, KH, 256], BF16, kind="ExternalOutput")
        taps["H0"] = nc.dram_tensor("tap_H0", [CM, KH, 8], BF16, kind="ExternalOutput")

    with tile.TileContext(nc) as tc, ExitStack() as ctx:
        const = ctx.enter_context(tc.tile_pool(name="const", bufs=1))
        acts = ctx.enter_context(tc.tile_pool(name="acts", bufs=1))
        tr = ctx.enter_context(tc.tile_pool(name="tr", bufs=1))
        sm = ctx.enter_context(tc.tile_pool(name="sm", bufs=1))

        # ---- input DMAs split across both HWDGE rings by consumption
        # order. Sync ring carries the critical first-leaf inputs; the
        # scalar ring first runs the ACT-table warmup (so the table loads
        # overlap the sync-ring transfers), then issues the later inputs.
        embl_t = acts.tile([128, 2, KE, 512], BF16, tag="embl", name="embl")
        wbig_t = const.tile([128, 4, KE, KH * CM], BF16, tag="wbig", name="wbig")
        embi_t = acts.tile([128, KE, NINTP], BF16, tag="embi", name="embi")
        ubig_t = const.tile([CH, KH, 4 * KH * CM + 2], BF16, tag="ubig", name="ubig")
        bias_t = const.tile([2, 1], F32, tag="bias", name="bias")
        nc.sync.dma_start(embl_t[:, 0], embl_d[:, 0])
        nc.sync.dma_start(wbig_t[:, 0], wbig_d[:, 0])
        nc.sync.dma_start(wbig_t[:, 2], wbig_d[:, 2])
        nc.sync.dma_start(embl_t[:, 1], embl_d[:, 1])
        nc.sync.dma_start(wbig_t[:, 1], wbig_d[:, 1])
        nc.sync.dma_start(wbig_t[:, 3], wbig_d[:, 3])
        nc.sync.dma_start(embi_t[:], embi_d[:])
        nc.sync.dma_start(ubig_t[:], ubig_d[:])
        nc.sync.dma_start(bias_t[:], bias_d[:])

        ones_t = const.tile([1, BL], BF16)
        nc.vector.memset(ones_t[:], 1.0)
        dummy_t = const.tile([1, BL], BF16)
        nc.scalar.activation(dummy_t[:], ones_t[:], AF.Sigmoid)

        def wiou_ap(g, k, m):  # stationary [128, 128]
            return wbig_t[:, g, k, CM * m : CM * (m + 1)]

        def wf_ap(k, m):
            return wbig_t[:, 3, k, CM * m : CM * (m + 1)]

        def uiou_ap(g, k, m):  # stationary [100, 128]
            return ubig_t[:, k, KH * CM * g + CM * m : KH * CM * g + CM * (m + 1)]

        def uf_ap(k, m):
            return ubig_t[:, k, 3 * KH * CM + CM * m : 3 * KH * CM + CM * (m + 1)]

        def wz_ap(k):  # stationary [100, 2]
            return ubig_t[:, k, 4 * KH * CM : 4 * KH * CM + 2]

        # ---- PE warm-up on a memset tile: release the HAM clock gate
        # (PE defaults to 1.2GHz; ~3.4us of sustained activity unlocks
        # 2.4GHz) while the input DMAs stream in.
        wtile = const.tile([128, 512], BF16, tag="wtile", name="wtile")
        nc.vector.memset(wtile[:], 0.0)

        # leaf gate tiles [CH, 3(mchunk), 128(node), 8(batch)]
        gi7 = acts.tile([CM, 3, 128, 8], BF16, tag="gi7")
        go7 = acts.tile([CM, 3, 128, 8], BF16, tag="go7")
        gu7 = acts.tile([CM, 3, 128, 8], BF16, tag="gu7")
        leaf_g = (gi7, go7, gu7)
        wxf_t = acts.tile([CM, KH, NINTP], BF16, tag="wxf")
        # W_iou emb + b for all of levels 3..0 (embi cols 8..136), computed
        # during the leaf-phase PE idle window; levels <=3 preload their iou
        # psums from this on DVE instead of running 18 prologue matmuls.
        wx3_t = acts.tile([CM, 9, 128], BF16, tag="wx3")

        with ExitStack() as ps1:
            ps_wx = ps1.enter_context(
                tc.tile_pool(name="ps_wx", bufs=2, space="PSUM")
            )

            # Bridge PE activity from engine-start to the first leaf matmul
            # (wiou g0 + embl half 0 land ~2us in) so the HAM busy window
            # stays warm and unlocks 2.4GHz ASAP.
            warm_ps = ps_wx.tile([CM, 3, 64, 8], F32, tag="ps_wx", name="warm")
            for w in range(6):
                nc.tensor.matmul(
                    warm_ps[:, w % 3, :, :],
                    wtile[:, 0:CM],
                    wtile[:],
                    start=(w < 3),
                    stop=(w >= 3),
                    skip_group_check=True,
                )

            # ---- leaf gates fused with wx matmuls (bias folded into W row
            # 200 x emb bias channel == 1). One merged [CH, 3, 512] psum per
            # (half, gate) -> one ACT per gate half.
            for nt in range(2):
                for g in (0, 2, 1):
                    ps = ps_wx.tile([CM, 3, 64, 8], F32, tag="ps_wx", name="psw")
                    for m in range(3):
                        for k in range(KE):
                            nc.tensor.matmul(
                                ps[:, m, :, :],
                                wiou_ap(g, k, m),
                                embl_t[:, nt, k, :],
                                start=(k == 0),
                                stop=(k == KE - 1),
                            )
                    nc.scalar.activation(
                        leaf_g[g][:, :, 64 * nt : 64 * (nt + 1), :],
                        ps[:],
                        AF.Tanh if g == 2 else AF.Sigmoid,
                    )

            # ---- internal wxf (level 6 consumes it first; deep half first)
            for nt in (1, 0):
                ps = ps_wx.tile([CM, 3, 512], F32, tag="ps_wx", name="psw")
                for m in range(KH):
                    for k in range(KE):
                        nc.tensor.matmul(
                            ps[:, m, :],
                            wf_ap(k, m),
                            embi_t[:, k, 512 * nt : 512 * (nt + 1)],
                            start=(k == 0),
                            stop=(k == KE - 1),
                        )
                nc.vector.tensor_copy(
                    wxf_t[:, :, 512 * nt : 512 * (nt + 1)], ps[:]
                )

            # ---- wx3: W_iou emb[par] + b for levels 3..0 ----
            ps = ps_wx.tile([CM, 9, 128], F32, tag="ps_wx", name="psw3")
            for m9 in range(9):
                for k in range(KE):
                    nc.tensor.matmul(
                        ps[:, m9, :],
                        wiou_ap(m9 // 3, k, m9 % 3),
                        embi_t[:, k, 8:136],
                        start=(k == 0),
                        stop=(k == KE - 1),
                    )
            nc.vector.tensor_copy(wx3_t[:], ps[:])

        # ---- leaf cell/hidden ----
        H = {}
        C = {}
        C[7] = acts.tile([CM, KH, 128, 8], BF16, tag="C7", name="C7")
        H[7] = acts.tile([CM, KH, 128, 8], BF16, tag="H7", name="H7")
        for h in range(2):
            nsl = slice(64 * h, 64 * (h + 1))
            nc.vector.tensor_mul(
                C[7][:, :, nsl, :], gi7[:, :, nsl, :], gu7[:, :, nsl, :]
            )
            th7 = tr.tile([CM, KH, 64, 8], BF16, tag="th", name="th7")
            nc.scalar.activation(th7[:], C[7][:, :, nsl, :], AF.Tanh)
            nc.vector.tensor_mul(
                H[7][:, :, nsl, :], go7[:, :, nsl, :], th7[:]
            )

        # ---- tree levels 6..0 ----
        def level_body(d, ps_uf, ps_io, all_pss):
            npar = 1 << d
            n = npar * 8
            off = 8 * (1 << d)  # internal idx layout: [pad(8), nodes]
            Hch, Cch = H[d + 1], C[d + 1]
            psn = min(n, 512)

            # iou psum: W_iou emb[par] + bias -- prologue matmuls for the
            # big levels, DVE preload from wx3 for d<=3.
            def iou_prologue(g):
                ps = ps_io.tile([CM, KH, psn], F32, tag="ps_io", name="psg")
                if d <= 3:
                    nc.vector.tensor_copy(
                        ps[:, :, 0:n],
                        wx3_t[:, :, off - 8 : off - 8 + n].rearrange(
                            "p (g3 m) c -> p g3 m c", m=KH
                        )[:, g],
                    )
                else:
                    for m in range(KH):
                        for k2 in range(KE):
                            nc.tensor.matmul(
                                ps[:, m, 0:n],
                                wiou_ap(g, k2, m),
                                embi_t[:, k2, off : off + n],
                                start=(k2 == 0),
                                stop=False,
                            )
                return ps

            if all_pss:
                pss = {g: iou_prologue(g) for g in (0, 2, 1)}
            else:
                pss = {0: iou_prologue(0), 2: iou_prologue(2)}

            hsum = tr.tile([CM, KH, npar, 8], BF16, tag="hsum", name="hsum")
            for k in range(KH):
                nc.vector.tensor_add(
                    hsum[:, k], Hch[:, k, 0::2, :], Hch[:, k, 1::2, :]
                )

            # forget gates: psum preloaded with wxf (vector cast), U_f
            # matmuls accumulate; f = sigma(psum). [CH, KH, npar, 2, 8].
            f2 = tr.tile([CM, KH, npar, 2, 8], BF16, tag="f2", name="f2")
            nchunk = max(n // 256, 1)  # child cols per psum <= 512
            cpn = npar // nchunk
            for m in range(KH):
                for h in range(nchunk):
                    ps = ps_uf.tile(
                        [CM, cpn, 2, 8], F32, tag="ps_uf", name="psu"
                    )
                    wxfs = wxf_t[
                        :, m, off + 8 * cpn * h : off + 8 * cpn * (h + 1)
                    ].rearrange("p (n e) -> p n e", e=8)
                    nc.vector.tensor_copy(ps[:, :, 0, :], wxfs)
                    nc.vector.tensor_copy(ps[:, :, 1, :], wxfs)
                    for k in range(KH):
                        nc.tensor.matmul(
                            ps[:],
                            uf_ap(k, m),
                            Hch[0:CH, k, 2 * cpn * h : 2 * cpn * (h + 1), :],
                            start=False,
                            stop=(k == KH - 1),
                            skip_group_check=True,
                        )
                    nc.scalar.activation(
                        f2[:, m, cpn * h : cpn * (h + 1), :, :],
                        ps[:],
                        AF.Sigmoid,
                    )
            # fc = sum over the two children of f * C (vector, bf16)
            tci2 = tr.tile([CM, KH, npar, 2, 8], BF16, tag="tci2", name="tci2")
            cview = Cch[:].rearrange("p k (n two) b -> p k n two b", two=2)
            fc = tr.tile([CM, KH, npar, 8], BF16, tag="fc", name="fc")
            nh2 = 2 if npar >= 32 else 1
            hp2 = npar // nh2
            for q in range(nh2):
                s_ = slice(hp2 * q, hp2 * (q + 1))
                nc.vector.tensor_mul(
                    tci2[:, :, s_, :, :], f2[:, :, s_, :, :],
                    cview[:, :, s_, :, :],
                )
                nc.vector.tensor_add(
                    fc[:, :, s_, :], tci2[:, :, s_, 0, :],
                    tci2[:, :, s_, 1, :],
                )

            # iou gates: psum = W_iou emb[par] + b + U_iou hsum; one ACT
            # per gate over all three m-chunks.
            gates = {}
            for g in (0, 2, 1):
                gt = tr.tile(
                    [CM, KH, npar, 8], BF16, tag=f"g{g}", name="gt"
                )
                ps = pss.get(g)
                if ps is None:
                    ps = iou_prologue(g)
                for m in range(KH):
                    for k in range(KH):
                        nc.tensor.matmul(
                            ps[:, m, 0:n],
                            uiou_ap(g, k, m),
                            hsum[0:CH, k, :, :],
                            start=False,
                            stop=(k == KH - 1),
                            skip_group_check=True,
                        )
                nc.scalar.activation(
                    gt[:],
                    ps[:, :, 0:n].rearrange("p m (c b) -> p m c b", b=8),
                    AF.Tanh if g == 2 else AF.Sigmoid,
                )
                gates[g] = gt
            gi, go, gu = gates[0], gates[1], gates[2]

            # HAM keep-warm fillers: run while vector/scalar do the
            # level tail; next level's matmuls then start at 2.4GHz.
            fill_ps = ps_uf.tile(
                [CM, 512], F32, tag="ps_uf", name="fill"
            )
            for w in range(8 if npar >= 8 else 4):
                nc.tensor.matmul(
                    fill_ps[:],
                    wtile[:, 0:CM],
                    wtile[:],
                    start=(w == 0),
                    stop=False,
                    skip_group_check=True,
                )

            tci = tr.tile([CM, KH, npar, 8], BF16, tag="tci", name="tci")
            C[d] = acts.tile(
                [CM, KH, npar, 8], BF16, tag=f"C{d}", name=f"C{d}"
            )
            sc = tr.tile([CM, KH, npar, 8], BF16, tag="sc", name="sc")
            H[d] = acts.tile(
                [CM, KH, npar, 8], BF16, tag=f"H{d}", name=f"H{d}"
            )
            # split the elementwise tail into node-halves so the
            # vector/scalar chain pipelines (big levels only).
            nh = 2 if npar >= 32 else 1
            hp = npar // nh
            for q in range(nh):
                s_ = slice(hp * q, hp * (q + 1))
                nc.vector.tensor_mul(
                    tci[:, :, s_, :], gi[:, :, s_, :], gu[:, :, s_, :]
                )
                nc.vector.tensor_add(
                    C[d][:, :, s_, :], tci[:, :, s_, :], fc[:, :, s_, :]
                )
                nc.scalar.activation(
                    sc[:, :, s_, :], C[d][:, :, s_, :], AF.Tanh
                )
                nc.vector.tensor_mul(
                    H[d][:, :, s_, :], go[:, :, s_, :], sc[:, :, s_, :]
                )

        with ExitStack() as ps2:
            ps_uf = ps2.enter_context(
                tc.tile_pool(name="ps_uf", bufs=2, space="PSUM")
            )
            ps_io = ps2.enter_context(
                tc.tile_pool(name="ps_io", bufs=2, space="PSUM")
            )
            level_body(6, ps_uf, ps_io, all_pss=False)

        with ExitStack() as ps3:
            ps_uf2 = ps3.enter_context(
                tc.tile_pool(name="ps_uf2", bufs=2, space="PSUM")
            )
            ps_io2 = ps3.enter_context(
                tc.tile_pool(name="ps_io2", bufs=3, space="PSUM")
            )
            for d in range(5, -1, -1):
                level_body(d, ps_uf2, ps_io2, all_pss=True)

            # ---- head: psum rows = [z', -z'] (z = z' + bz);
            # log_softmax = [ln s(z), ln s(-z)] -> sigmoid ACT (bias
            # [+bz, -bz]) then Ln ACT (natural_log table), transpose on host.
            ps = ps_io2.tile([2, BL], F32, tag="ps_io", name="pshead")
            for k in range(KH):
                nc.tensor.matmul(
                    ps[:],
                    wz_ap(k),
                    H[0][0:CH, k, 0, :],
                    start=(k == 0),
                    stop=(k == KH - 1),
                )
            sg_t = sm.tile([2, BL], F32, tag="sg")
            nc.scalar.activation(sg_t[:], ps[:], AF.Sigmoid, bias=bias_t[:])
            ln_t = sm.tile([2, BL], F32, tag="ln")
            nc.scalar.activation(ln_t[:], sg_t[:], AF.Ln)
            nc.sync.dma_start(out_d[:], ln_t[:])

        # ---- debug taps ----
        if TAPS:
            nc.sync.dma_start(taps["wxf"][:], wxf_t[:])
            nc.sync.dma_start(taps["H7"][:], H[7][:])
            nc.sync.dma_start(taps["C7"][:], C[7][:])
            nc.sync.dma_start(taps["H5"][:], H[5][:])
            nc.sync.dma_start(taps["H0"][:], H[0][:])

    nc.compile()
    return nc


_CACHE = {}


def _get_nc():
    if "nc" not in _CACHE:
        _CACHE["nc"] = _build()
    return _CACHE["nc"]


def _mpad(w, cols):
    """Pad each 100-wide output chunk of w [rows, cols] to 128 (zero cols)
    so stationary tiles are full 128 columns (FWL eligible)."""
    rows = w.shape[0]
    nm = cols // CH
    p = np.zeros((rows, nm, CM), np.float32)
    p[:, :, :CH] = w.reshape(rows, nm, CH)
    return p.reshape(rows, nm * CM)


def kernel(x, parent, depth, embed, W_iou, U_iou, b_iou, W_f, U_f, b_f,
           W_out, b_out):
    x = np.asarray(x)
    embed = np.asarray(embed, dtype=np.float32)
    # feature-major bf16 embedding table with bias channel at row E
    embT = np.zeros((256, V), BF16NP)
    embT[:E] = embed.T.astype(BF16NP)
    embT[E] = 1.0

    W_out = np.asarray(W_out, np.float32)
    b_out = np.asarray(b_out, np.float32)
    wz = W_out[:, 0] - W_out[:, 1]  # [300]
    bz = float(b_out[0] - b_out[1])
    wz2 = np.stack([wz, -wz], axis=1)  # [300, 2]

    wiou_b = np.vstack(
        [np.asarray(W_iou, np.float32), np.asarray(b_iou, np.float32)[None]]
    )  # [201, 900]
    wf_b = np.vstack(
        [np.asarray(W_f, np.float32), np.asarray(b_f, np.float32)[None]]
    )  # [201, 300]

    def kpad(w, rows_to):
        p = np.zeros((rows_to, w.shape[1]), np.float32)
        p[: w.shape[0]] = w
        return p

    wiou_p = kpad(_mpad(wiou_b, 3 * HID), 256)  # [256, 1152]
    wf_p = kpad(_mpad(wf_b, HID), 256)  # [256, 384]
    uiou_p = _mpad(np.asarray(U_iou, np.float32), 3 * HID)  # [300, 1152]
    uf_p = _mpad(np.asarray(U_f, np.float32), HID)  # [300, 384]

    # wbig [128, 4(g), KE, 384]: wiou gates 0..2, wf as gate 3
    wiou_r = wiou_p.reshape(2, 128, 3, KH * CM).transpose(1, 2, 0, 3)
    wf_r = wf_p.reshape(2, 128, 1, KH * CM).transpose(1, 2, 0, 3)
    wbig = np.concatenate([wiou_r, wf_r], axis=1)  # [128, 4, 2, 384]
    # ubig [100, KH(k), 4*384+2]: per k: [uiou g0|g1|g2, uf, wz]
    uiou_r = uiou_p.reshape(3, 100, 3, KH * CM).transpose(1, 0, 2, 3)  # [100,k,g,384]
    uiou_r = uiou_r.reshape(100, 3, 3 * KH * CM)
    uf_r = uf_p.reshape(3, 100, KH * CM).transpose(1, 0, 2)  # [100, k, 384]
    wz_r = wz2.reshape(3, 100, 2).transpose(1, 0, 2)  # [100, k, 2]
    ubig = np.concatenate([uiou_r, uf_r, wz_r], axis=2)  # [100, 3, 1538]
    shared = {
        "wbig": np.ascontiguousarray(wbig).astype(BF16NP),
        "ubig": np.ascontiguousarray(ubig).astype(BF16NP),
        "bias": np.array([[bz], [-bz]], np.float32),
    }
    in_maps = []
    for c in range(NCORES):
        xc = x[:, BL * c : BL * (c + 1)]  # [255, 8]
        leaf = np.ascontiguousarray(xc[127:255]).reshape(-1)  # 1024
        internal = np.concatenate(
            [np.zeros(8, np.int64), np.ascontiguousarray(xc[0:127]).reshape(-1)]
        )
        im = dict(shared)
        im["embl"] = np.ascontiguousarray(
            embT[:, leaf].reshape(2, 128, 2, 512).transpose(1, 2, 0, 3)
        )
        im["embi"] = np.ascontiguousarray(
            embT[:, internal].reshape(2, 128, NINTP).transpose(1, 0, 2)
        )
        in_maps.append(im)

    nc = _get_nc()
    res = run_bass_kernel_spmd(nc, in_maps, core_ids=list(range(NCORES)))
    kernel._last = res
    out = np.concatenate(
        [np.asarray(res.results[c]["out"]).T for c in range(NCORES)], axis=0
    )
    return np.ascontiguousarray(out.astype(np.float32))


kernel._last = None
